# revision 21
# baseline (speedup 1.0000x reference)
"""Trainium2 Bass kernel: LIF step + STDP correlation sensors (batch-data-parallel over 8 cores).

Shapes are hardcoded for B=32, IN=H=512. Each core processes 4 batches; the
small weight matrices are replicated (pre-transposed on the host so the
TensorEngine can consume them directly as the matmul rhs).
"""

import numpy as np

B, IN, H = 32, 512, 512
NCORES = 8
BPC = B // NCORES  # batches per core
P = 128            # SBUF partitions
T = H // P         # row chunks folded into the free dim (4)

DT = 0.001
DECAY = float(1.0 - DT * 10.0)   # corr/anti decay (1 - DT*TAU_C_INV) = 0.99
I_DEC = float(-DT * 200.0)       # -DT*TAU_SYN_INV = -0.2
V_COEF = float(DT * 100.0)       # DT*TAU_MEM_INV = 0.1
V_TH = 1.0

_CACHE = {}

BIG = [(pfx, sfx) for pfx in ("ic", "rc") for sfx in ("pp", "cr", "an")]


def _emit(nc, tc, d, use_bcast_ap=True):
    import concourse.mybir as mybir

    f32 = mybir.dt.float32
    Alu = mybir.AluOpType

    with tc.tile_pool(name="small", bufs=1) as small, \
         tc.tile_pool(name="wpool", bufs=1) as wpool, \
         tc.tile_pool(name="pbc", bufs=2, space="PSUM") as pbc, \
         tc.tile_pool(name="mmp", bufs=1, space="PSUM") as mmp, \
         tc.tile_pool(name="io", bufs=4) as io, \
         tc.tile_pool(name="tmp", bufs=2) as tmp:

        pairs = [(b, pfx) for b in range(BPC) for pfx in ("ic", "rc")]

        def load_pair(b, pfx):
            src = {sfx: d[f"{pfx}_{sfx}"][b].rearrange("(p t) n -> p t n", p=P) for sfx in ("pp", "cr", "an")}
            tiles = {}
            for sfx in ("pp", "cr", "an"):
                t_ = io.tile([P, T, H], f32, name=sfx, tag=sfx)
                nc.sync.dma_start(out=t_[:], in_=src[sfx])
                tiles[sfx] = t_
            return tiles

        # Prefetch the first pair's 3MB before anything else so the DMA
        # engines saturate from t=0 (tiny loads slot in behind on other rings).
        prefetched = {0: load_pair(*pairs[0])}

        # ---------------- small loads ----------------
        v_sb = small.tile([BPC, H], f32, name="v_sb")
        nc.sync.dma_start(out=v_sb[:], in_=d["v"])
        i_sb = small.tile([BPC, H], f32, name="i_sb")
        nc.sync.dma_start(out=i_sb[:], in_=d["i"])
        sel_sb = small.tile([BPC, BPC, P], f32, name="sel_sb")
        nc.sync.dma_start(out=sel_sb[:], in_=d["selmat"])
        # xT/zT: [512, BPC] -> [P, (t b)] so that column c*BPC+b is the
        # per-partition pre-spike vector for (batch b, row-chunk c).
        xT_sb = small.tile([P, T * BPC], f32, name="xT_sb")
        nc.sync.dma_start(out=xT_sb[:], in_=d["xT"].rearrange("(p t) b -> p (t b)", p=P))
        zT_sb = small.tile([P, T * BPC], f32, name="zT_sb")
        nc.sync.dma_start(out=zT_sb[:], in_=d["zT"].rearrange("(p t) b -> p (t b)", p=P))

        # derived pre columns: -pre and (1-pre)
        negx = small.tile([P, T * BPC], f32, name="negx")
        nc.vector.tensor_scalar(out=negx[:], in0=xT_sb[:], scalar1=-1.0, scalar2=None, op0=Alu.mult)
        notx = small.tile([P, T * BPC], f32, name="notx")
        nc.vector.tensor_scalar(out=notx[:], in0=xT_sb[:], scalar1=-1.0, scalar2=1.0, op0=Alu.mult, op1=Alu.add)
        negz = small.tile([P, T * BPC], f32, name="negz")
        nc.vector.tensor_scalar(out=negz[:], in0=zT_sb[:], scalar1=-1.0, scalar2=None, op0=Alu.mult)
        notz = small.tile([P, T * BPC], f32, name="notz")
        nc.vector.tensor_scalar(out=notz[:], in0=zT_sb[:], scalar1=-1.0, scalar2=1.0, op0=Alu.mult, op1=Alu.add)

        for k in (1, 2):
            prefetched[k] = load_pair(*pairs[k])

        # ---------------- LIF step (exact op order vs reference) ----------------
        dtmp = small.tile([BPC, H], f32, name="dtmp")
        nc.vector.tensor_sub(dtmp[:], i_sb[:], v_sb[:])          # (V_LEAK - v) + i == i - v
        nc.vector.tensor_scalar_mul(dtmp[:], dtmp[:], V_COEF)    # 0.1 * (i - v)
        vd = small.tile([BPC, H], f32, name="vd")
        nc.vector.tensor_add(vd[:], v_sb[:], dtmp[:])            # v_decayed
        zn = small.tile([BPC, H], f32, name="zn")
        nc.vector.tensor_single_scalar(zn[:], vd[:], V_TH, Alu.is_gt)  # heaviside(vd - 1)
        znot = small.tile([BPC, H], f32, name="znot")
        nc.vector.tensor_scalar(out=znot[:], in0=zn[:], scalar1=-1.0, scalar2=1.0, op0=Alu.mult, op1=Alu.add)
        vn = small.tile([BPC, H], f32, name="vn")
        nc.vector.tensor_mul(vn[:], vd[:], znot[:])              # v_new (V_RESET = 0)

        # ---------------- broadcast z_new across partitions ----------------
        # out = sel_b.T @ zn with sel_b[k, :] = (k == b) replicates row b of zn
        # onto all 128 partitions (PE rhs must sit at base partition 0).
        posts = []
        for b in range(BPC):
            pb = pbc.tile([P, H], f32, name="pb", tag="pb")
            nc.tensor.matmul(pb[:], lhsT=sel_sb[:, b, :], rhs=zn[:, :], start=True, stop=True)
            post = small.tile([P, H], f32, name=f"post{b}")
            nc.scalar.copy(post[:], pb[:])
            posts.append(post)

        # weights (already transposed on host): rhs chunks with k = 4p+c row order
        wts = []
        for wname in ("wt_in", "wt_rec"):
            chunks = []
            for c in range(T):
                wt = wpool.tile([P, H], f32, name=f"{wname}_{c}")
                nc.sync.dma_start(out=wt[:], in_=d[wname].rearrange("(p t) h -> t p h", p=P)[c])
                chunks.append(wt)
            wts.append(chunks)

        # ---------------- i_new = i_decayed + x @ Win.T + z @ Wrec.T ----------------
        idec = small.tile([BPC, H], f32, name="idec")
        nc.vector.tensor_scalar_mul(idec[:], i_sb[:], I_DEC)     # -0.2 * i
        nc.vector.tensor_add(idec[:], i_sb[:], idec[:])          # i_decayed
        psum_i = mmp.tile([BPC, H], f32, name="psum_i")
        k = 0
        for colT, chunks in ((xT_sb, wts[0]), (zT_sb, wts[1])):
            for c in range(T):
                nc.tensor.matmul(
                    psum_i[:],
                    lhsT=colT[:, c * BPC:(c + 1) * BPC],
                    rhs=chunks[c][:],
                    start=(k == 0),
                    stop=(k == 2 * T - 1),
                )
                k += 1
        inew = small.tile([BPC, H], f32, name="inew")
        nc.vector.tensor_add(inew[:], idec[:], psum_i[:])

        # ---------------- correlation sensors: stream 1MB tiles ----------------
        for idx, (b, pfx) in enumerate(pairs):
            tiles = prefetched.pop(idx)
            if idx + 3 < len(pairs):
                prefetched[idx + 3] = load_pair(*pairs[idx + 3])
            negc, notc, prec = (negx, notx, xT_sb) if pfx == "ic" else (negz, notz, zT_sb)
            dst = {sfx: d[f"o_{pfx}_{sfx}"][b].rearrange("(p t) n -> p t n", p=P) for sfx in ("pp", "cr", "an")}
            pp, cr, an = tiles["pp"], tiles["cr"], tiles["an"]

            if True:
                t1 = tmp.tile([P, T, H], f32, name="t1", tag="t1")  # post * post_pre
                if use_bcast_ap:
                    nc.vector.tensor_mul(t1[:], pp[:], posts[b][:].unsqueeze(1).to_broadcast((P, T, H)))
                else:
                    for c in range(T):
                        nc.vector.tensor_mul(t1[:, c, :], pp[:, c, :], posts[b][:])
                q = tmp.tile([P, T, H], f32, name="q", tag="q")     # (1-post) * post_pre
                nc.vector.tensor_sub(q[:], pp[:], t1[:])
                t2 = tmp.tile([P, T, H], f32, name="t2", tag="t2")  # pre * (1 - post_pre)
                for c in range(T):
                    col = slice(c * BPC + b, c * BPC + b + 1)
                    nc.vector.tensor_scalar(
                        out=t2[:, c, :], in0=pp[:, c, :],
                        scalar1=negc[:, col], scalar2=prec[:, col],
                        op0=Alu.mult, op1=Alu.add,
                    )
                    # post_pre_new = (1-pre)*q + pre, in place on q
                    nc.vector.tensor_scalar(
                        out=q[:, c, :], in0=q[:, c, :],
                        scalar1=notc[:, col], scalar2=prec[:, col],
                        op0=Alu.mult, op1=Alu.add,
                    )
                nc.sync.dma_start(out=dst["pp"], in_=q[:])

                crs = tmp.tile([P, T, H], f32, name="crs", tag="crs")
                nc.scalar.mul(crs[:], cr[:], DECAY)                 # 0.99 * corr on ScalarE
                nc.vector.tensor_add(crs[:], crs[:], t1[:])         # + post*post_pre
                nc.sync.dma_start(out=dst["cr"], in_=crs[:])

                ans = tmp.tile([P, T, H], f32, name="ans", tag="t1")
                nc.scalar.mul(ans[:], an[:], DECAY)                 # 0.99 * anti
                nc.vector.tensor_add(ans[:], ans[:], t2[:])         # + pre*(1-post_pre)
                nc.sync.dma_start(out=dst["an"], in_=ans[:])

        # Small stores issued last: they must not head-of-line-block the big
        # streaming loads in the DMA issue queue (zn/vn/inew stay live in the
        # bufs=1 "small" pool).
        nc.sync.dma_start(out=d["z_new"], in_=zn[:])
        nc.sync.dma_start(out=d["v_new"], in_=vn[:])
        nc.sync.dma_start(out=d["i_new"], in_=inew[:])


def build(use_bcast_ap=True):
    """Build + bacc-compile the single-core SPMD Bass module."""
    import concourse.bacc as bacc
    import concourse.mybir as mybir
    import concourse.tile as tile

    f32 = mybir.dt.float32
    nc = bacc.Bacc("TRN2", target_bir_lowering=False, debug=False, enable_asserts=False)

    d = {}
    for name, shape in (
        ("v", (BPC, H)), ("i", (BPC, H)),
        ("xT", (IN, BPC)), ("zT", (H, BPC)),
        ("wt_in", (IN, H)), ("wt_rec", (H, H)),
        ("selmat", (BPC, BPC, P)),
    ):
        d[name] = nc.dram_tensor(name, shape, f32, kind="ExternalInput").ap()
    for pfx, sfx in BIG:
        d[f"{pfx}_{sfx}"] = nc.dram_tensor(f"{pfx}_{sfx}", (BPC, IN if pfx == "ic" else H, H), f32, kind="ExternalInput").ap()
        d[f"o_{pfx}_{sfx}"] = nc.dram_tensor(f"o_{pfx}_{sfx}", (BPC, IN if pfx == "ic" else H, H), f32, kind="ExternalOutput").ap()
    for name in ("z_new", "v_new", "i_new"):
        d[name] = nc.dram_tensor(name, (BPC, H), f32, kind="ExternalOutput").ap()

    with tile.TileContext(nc) as tc:
        _emit(nc, tc, d, use_bcast_ap=use_bcast_ap)
    nc.compile()
    return nc


def _get_nc():
    if "nc" not in _CACHE:
        _CACHE["nc"] = build()
    return _CACHE["nc"]


def make_in_maps(inputs):
    """Split full-size inputs into 8 per-core input maps."""
    f = lambda a: np.ascontiguousarray(np.asarray(a), dtype=np.float32)
    src = {
        "ic_pp": f(inputs["ic_post_pre"]), "ic_cr": f(inputs["ic_corr"]), "ic_an": f(inputs["ic_anti"]),
        "rc_pp": f(inputs["rc_post_pre"]), "rc_cr": f(inputs["rc_corr"]), "rc_an": f(inputs["rc_anti"]),
    }
    x = f(inputs["input"]); z = f(inputs["z"]); v = f(inputs["v"]); i = f(inputs["i"])
    wt_in = np.ascontiguousarray(f(inputs["input_weights"]).T)
    wt_rec = np.ascontiguousarray(f(inputs["recurrent_weights"]).T)
    selmat = np.ascontiguousarray(
        np.repeat(np.eye(BPC, dtype=np.float32)[:, :, None], P, axis=2))
    in_maps = []
    for c in range(NCORES):
        sl = slice(c * BPC, (c + 1) * BPC)
        m = {
            "v": np.ascontiguousarray(v[sl]),
            "i": np.ascontiguousarray(i[sl]),
            "xT": np.ascontiguousarray(x[sl].T),
            "zT": np.ascontiguousarray(z[sl].T),
            "wt_in": wt_in,
            "wt_rec": wt_rec,
            "selmat": selmat,
        }
        for k, a in src.items():
            m[k] = np.ascontiguousarray(a[sl])
        in_maps.append(m)
    return in_maps


def gather(results):
    """Concatenate 8 per-core output dicts into the reference's 9-tuple."""
    cat = lambda name: np.concatenate([np.asarray(results[c][name]) for c in range(NCORES)], axis=0)
    return (
        cat("z_new"), cat("v_new"), cat("i_new"),
        cat("o_ic_pp"), cat("o_ic_cr"), cat("o_ic_an"),
        cat("o_rc_pp"), cat("o_rc_cr"), cat("o_rc_an"),
    )


def kernel(**inputs):
    from concourse.bass_utils import run_bass_kernel_spmd

    nc = _get_nc()
    in_maps = make_in_maps(inputs)
    res = run_bass_kernel_spmd(nc, in_maps, core_ids=list(range(NCORES)))
    return gather(res.results)


# revision 22
# speedup vs baseline: 1.0025x; 1.0025x over previous
"""Trainium2 Bass kernel: LIF step + STDP correlation sensors (batch-data-parallel over 8 cores).

Shapes are hardcoded for B=32, IN=H=512. Each core processes 4 batches; the
small weight matrices are replicated (pre-transposed on the host so the
TensorEngine can consume them directly as the matmul rhs).
"""

import numpy as np

B, IN, H = 32, 512, 512
NCORES = 8
BPC = B // NCORES  # batches per core
P = 128            # SBUF partitions
T = H // P         # row chunks folded into the free dim (4)

DT = 0.001
DECAY = float(1.0 - DT * 10.0)   # corr/anti decay (1 - DT*TAU_C_INV) = 0.99
I_DEC = float(-DT * 200.0)       # -DT*TAU_SYN_INV = -0.2
V_COEF = float(DT * 100.0)       # DT*TAU_MEM_INV = 0.1
V_TH = 1.0

_CACHE = {}

BIG = [(pfx, sfx) for pfx in ("ic", "rc") for sfx in ("pp", "cr", "an")]


def _emit(nc, tc, d, use_bcast_ap=True):
    import concourse.mybir as mybir

    f32 = mybir.dt.float32
    Alu = mybir.AluOpType

    with tc.tile_pool(name="small", bufs=1) as small, \
         tc.tile_pool(name="wpool", bufs=1) as wpool, \
         tc.tile_pool(name="pbc", bufs=2, space="PSUM") as pbc, \
         tc.tile_pool(name="mmp", bufs=1, space="PSUM") as mmp, \
         tc.tile_pool(name="io", bufs=4) as io, \
         tc.tile_pool(name="tmp", bufs=2) as tmp:

        pairs = [(b, pfx) for b in range(BPC) for pfx in ("ic", "rc")]

        def load_pair(b, pfx):
            src = {sfx: d[f"{pfx}_{sfx}"][b].rearrange("(p t) n -> p t n", p=P) for sfx in ("pp", "cr", "an")}
            tiles = {}
            for sfx in ("pp", "cr", "an"):
                t_ = io.tile([P, T, H], f32, name=sfx, tag=sfx)
                nc.sync.dma_start(out=t_[:], in_=src[sfx])
                tiles[sfx] = t_
            return tiles

        # Prefetch the first pair's 3MB before anything else so the DMA
        # engines saturate from t=0 (tiny loads slot in behind on other rings).
        prefetched = {0: load_pair(*pairs[0])}

        # ---------------- small loads ----------------
        v_sb = small.tile([BPC, H], f32, name="v_sb")
        nc.sync.dma_start(out=v_sb[:], in_=d["v"])
        i_sb = small.tile([BPC, H], f32, name="i_sb")
        nc.sync.dma_start(out=i_sb[:], in_=d["i"])
        sel_sb = small.tile([BPC, BPC, P], f32, name="sel_sb")
        nc.sync.dma_start(out=sel_sb[:], in_=d["selmat"])
        # xT/zT: [512, BPC] -> [P, (t b)] so that column c*BPC+b is the
        # per-partition pre-spike vector for (batch b, row-chunk c).
        xT_sb = small.tile([P, T * BPC], f32, name="xT_sb")
        nc.sync.dma_start(out=xT_sb[:], in_=d["xT"].rearrange("(p t) b -> p (t b)", p=P))
        zT_sb = small.tile([P, T * BPC], f32, name="zT_sb")
        nc.sync.dma_start(out=zT_sb[:], in_=d["zT"].rearrange("(p t) b -> p (t b)", p=P))

        # derived pre columns: -pre and (1-pre)
        negx = small.tile([P, T * BPC], f32, name="negx")
        nc.vector.tensor_scalar(out=negx[:], in0=xT_sb[:], scalar1=-1.0, scalar2=None, op0=Alu.mult)
        notx = small.tile([P, T * BPC], f32, name="notx")
        nc.vector.tensor_scalar(out=notx[:], in0=xT_sb[:], scalar1=-1.0, scalar2=1.0, op0=Alu.mult, op1=Alu.add)
        negz = small.tile([P, T * BPC], f32, name="negz")
        nc.vector.tensor_scalar(out=negz[:], in0=zT_sb[:], scalar1=-1.0, scalar2=None, op0=Alu.mult)
        notz = small.tile([P, T * BPC], f32, name="notz")
        nc.vector.tensor_scalar(out=notz[:], in0=zT_sb[:], scalar1=-1.0, scalar2=1.0, op0=Alu.mult, op1=Alu.add)

        for k in (1, 2):
            prefetched[k] = load_pair(*pairs[k])

        # ---------------- LIF step (exact op order vs reference) ----------------
        dtmp = small.tile([BPC, H], f32, name="dtmp")
        nc.vector.tensor_sub(dtmp[:], i_sb[:], v_sb[:])          # (V_LEAK - v) + i == i - v
        nc.vector.tensor_scalar_mul(dtmp[:], dtmp[:], V_COEF)    # 0.1 * (i - v)
        vd = small.tile([BPC, H], f32, name="vd")
        nc.vector.tensor_add(vd[:], v_sb[:], dtmp[:])            # v_decayed
        zn = small.tile([BPC, H], f32, name="zn")
        nc.vector.tensor_single_scalar(zn[:], vd[:], V_TH, Alu.is_gt)  # heaviside(vd - 1)
        znot = small.tile([BPC, H], f32, name="znot")
        nc.vector.tensor_scalar(out=znot[:], in0=zn[:], scalar1=-1.0, scalar2=1.0, op0=Alu.mult, op1=Alu.add)
        vn = small.tile([BPC, H], f32, name="vn")
        nc.vector.tensor_mul(vn[:], vd[:], znot[:])              # v_new (V_RESET = 0)

        # ---------------- broadcast z_new across partitions ----------------
        # out = sel_b.T @ zn with sel_b[k, :] = (k == b) replicates row b of zn
        # onto all 128 partitions (PE rhs must sit at base partition 0).
        posts = []
        for b in range(BPC):
            pb = pbc.tile([P, H], f32, name="pb", tag="pb")
            nc.tensor.matmul(pb[:], lhsT=sel_sb[:, b, :], rhs=zn[:, :], start=True, stop=True)
            post = small.tile([P, H], f32, name=f"post{b}")
            nc.scalar.copy(post[:], pb[:])
            posts.append(post)

        # weights (already transposed on host): rhs chunks with k = 4p+c row order
        wts = []
        for wname in ("wt_in", "wt_rec"):
            chunks = []
            for c in range(T):
                wt = wpool.tile([P, H], f32, name=f"{wname}_{c}")
                nc.sync.dma_start(out=wt[:], in_=d[wname].rearrange("(p t) h -> t p h", p=P)[c])
                chunks.append(wt)
            wts.append(chunks)

        # ---------------- i_new = i_decayed + x @ Win.T + z @ Wrec.T ----------------
        idec = small.tile([BPC, H], f32, name="idec")
        nc.vector.tensor_scalar_mul(idec[:], i_sb[:], I_DEC)     # -0.2 * i
        nc.vector.tensor_add(idec[:], i_sb[:], idec[:])          # i_decayed
        psum_i = mmp.tile([BPC, H], f32, name="psum_i")
        k = 0
        for colT, chunks in ((xT_sb, wts[0]), (zT_sb, wts[1])):
            for c in range(T):
                nc.tensor.matmul(
                    psum_i[:],
                    lhsT=colT[:, c * BPC:(c + 1) * BPC],
                    rhs=chunks[c][:],
                    start=(k == 0),
                    stop=(k == 2 * T - 1),
                )
                k += 1
        inew = small.tile([BPC, H], f32, name="inew")
        nc.vector.tensor_add(inew[:], idec[:], psum_i[:])

        # ---------------- correlation sensors: stream 1MB tiles ----------------
        for idx, (b, pfx) in enumerate(pairs):
            tiles = prefetched.pop(idx)
            if idx + 3 < len(pairs):
                prefetched[idx + 3] = load_pair(*pairs[idx + 3])
            negc, notc, prec = (negx, notx, xT_sb) if pfx == "ic" else (negz, notz, zT_sb)
            dst = {sfx: d[f"o_{pfx}_{sfx}"][b].rearrange("(p t) n -> p t n", p=P) for sfx in ("pp", "cr", "an")}
            pp, cr, an = tiles["pp"], tiles["cr"], tiles["an"]

            if True:
                t1 = tmp.tile([P, T, H], f32, name="t1", tag="t1")  # post * post_pre
                if use_bcast_ap:
                    nc.vector.tensor_mul(t1[:], pp[:], posts[b][:].unsqueeze(1).to_broadcast((P, T, H)))
                else:
                    for c in range(T):
                        nc.vector.tensor_mul(t1[:, c, :], pp[:, c, :], posts[b][:])
                q = tmp.tile([P, T, H], f32, name="q", tag="q")     # (1-post) * post_pre
                nc.vector.tensor_sub(q[:], pp[:], t1[:])
                t2 = tmp.tile([P, T, H], f32, name="t2", tag="t2")  # pre * (1 - post_pre)
                for c in range(T):
                    col = slice(c * BPC + b, c * BPC + b + 1)
                    nc.vector.tensor_scalar(
                        out=t2[:, c, :], in0=pp[:, c, :],
                        scalar1=negc[:, col], scalar2=prec[:, col],
                        op0=Alu.mult, op1=Alu.add,
                    )
                    # post_pre_new = (1-pre)*q + pre, in place on q
                    nc.vector.tensor_scalar(
                        out=q[:, c, :], in0=q[:, c, :],
                        scalar1=notc[:, col], scalar2=prec[:, col],
                        op0=Alu.mult, op1=Alu.add,
                    )
                last = idx >= len(pairs) - 2
                if last:
                    # drain phase: chunk adds+stores so the final store is 256KB,
                    # not 1MB, letting the DMA tail finish right behind compute
                    for c in range(T):
                        nc.sync.dma_start(out=dst["pp"][:, c], in_=q[:, c, :])
                else:
                    nc.sync.dma_start(out=dst["pp"], in_=q[:])

                crs = tmp.tile([P, T, H], f32, name="crs", tag="crs")
                nc.scalar.mul(crs[:], cr[:], DECAY)                 # 0.99 * corr on ScalarE
                ans = tmp.tile([P, T, H], f32, name="ans", tag="t1")
                nc.scalar.mul(ans[:], an[:], DECAY)                 # 0.99 * anti
                if last:
                    for c in range(T):
                        nc.vector.tensor_add(crs[:, c, :], crs[:, c, :], t1[:, c, :])
                        nc.sync.dma_start(out=dst["cr"][:, c], in_=crs[:, c, :])
                    for c in range(T):
                        nc.vector.tensor_add(ans[:, c, :], ans[:, c, :], t2[:, c, :])
                        nc.sync.dma_start(out=dst["an"][:, c], in_=ans[:, c, :])
                else:
                    nc.vector.tensor_add(crs[:], crs[:], t1[:])     # + post*post_pre
                    nc.sync.dma_start(out=dst["cr"], in_=crs[:])
                    nc.vector.tensor_add(ans[:], ans[:], t2[:])     # + pre*(1-post_pre)
                    nc.sync.dma_start(out=dst["an"], in_=ans[:])

        # Small stores issued last: they must not head-of-line-block the big
        # streaming loads in the DMA issue queue (zn/vn/inew stay live in the
        # bufs=1 "small" pool).
        nc.sync.dma_start(out=d["z_new"], in_=zn[:])
        nc.sync.dma_start(out=d["v_new"], in_=vn[:])
        nc.sync.dma_start(out=d["i_new"], in_=inew[:])


def build(use_bcast_ap=True):
    """Build + bacc-compile the single-core SPMD Bass module."""
    import concourse.bacc as bacc
    import concourse.mybir as mybir
    import concourse.tile as tile

    f32 = mybir.dt.float32
    nc = bacc.Bacc("TRN2", target_bir_lowering=False, debug=False, enable_asserts=False)

    d = {}
    for name, shape in (
        ("v", (BPC, H)), ("i", (BPC, H)),
        ("xT", (IN, BPC)), ("zT", (H, BPC)),
        ("wt_in", (IN, H)), ("wt_rec", (H, H)),
        ("selmat", (BPC, BPC, P)),
    ):
        d[name] = nc.dram_tensor(name, shape, f32, kind="ExternalInput").ap()
    for pfx, sfx in BIG:
        d[f"{pfx}_{sfx}"] = nc.dram_tensor(f"{pfx}_{sfx}", (BPC, IN if pfx == "ic" else H, H), f32, kind="ExternalInput").ap()
        d[f"o_{pfx}_{sfx}"] = nc.dram_tensor(f"o_{pfx}_{sfx}", (BPC, IN if pfx == "ic" else H, H), f32, kind="ExternalOutput").ap()
    for name in ("z_new", "v_new", "i_new"):
        d[name] = nc.dram_tensor(name, (BPC, H), f32, kind="ExternalOutput").ap()

    with tile.TileContext(nc) as tc:
        _emit(nc, tc, d, use_bcast_ap=use_bcast_ap)
    nc.compile()
    return nc


def _get_nc():
    if "nc" not in _CACHE:
        _CACHE["nc"] = build()
    return _CACHE["nc"]


def make_in_maps(inputs):
    """Split full-size inputs into 8 per-core input maps."""
    f = lambda a: np.ascontiguousarray(np.asarray(a), dtype=np.float32)
    src = {
        "ic_pp": f(inputs["ic_post_pre"]), "ic_cr": f(inputs["ic_corr"]), "ic_an": f(inputs["ic_anti"]),
        "rc_pp": f(inputs["rc_post_pre"]), "rc_cr": f(inputs["rc_corr"]), "rc_an": f(inputs["rc_anti"]),
    }
    x = f(inputs["input"]); z = f(inputs["z"]); v = f(inputs["v"]); i = f(inputs["i"])
    wt_in = np.ascontiguousarray(f(inputs["input_weights"]).T)
    wt_rec = np.ascontiguousarray(f(inputs["recurrent_weights"]).T)
    selmat = np.ascontiguousarray(
        np.repeat(np.eye(BPC, dtype=np.float32)[:, :, None], P, axis=2))
    in_maps = []
    for c in range(NCORES):
        sl = slice(c * BPC, (c + 1) * BPC)
        m = {
            "v": np.ascontiguousarray(v[sl]),
            "i": np.ascontiguousarray(i[sl]),
            "xT": np.ascontiguousarray(x[sl].T),
            "zT": np.ascontiguousarray(z[sl].T),
            "wt_in": wt_in,
            "wt_rec": wt_rec,
            "selmat": selmat,
        }
        for k, a in src.items():
            m[k] = np.ascontiguousarray(a[sl])
        in_maps.append(m)
    return in_maps


def gather(results):
    """Concatenate 8 per-core output dicts into the reference's 9-tuple."""
    cat = lambda name: np.concatenate([np.asarray(results[c][name]) for c in range(NCORES)], axis=0)
    return (
        cat("z_new"), cat("v_new"), cat("i_new"),
        cat("o_ic_pp"), cat("o_ic_cr"), cat("o_ic_an"),
        cat("o_rc_pp"), cat("o_rc_cr"), cat("o_rc_an"),
    )


def kernel(**inputs):
    from concourse.bass_utils import run_bass_kernel_spmd

    nc = _get_nc()
    in_maps = make_in_maps(inputs)
    res = run_bass_kernel_spmd(nc, in_maps, core_ids=list(range(NCORES)))
    return gather(res.results)
